# revision 93
# baseline (speedup 1.0000x reference)
"""Bass/Trainium2 kernel for LocalAttention (block-diagonal MHA, causal).

Model: x[B=4, SEQ=4096, D=1024] split into SPLIT=4 sequence blocks of L=1024,
each with its own MHA weights (H=16 heads, DK=64), causal within block.

Sharding: 16 (batch, split) blocks over 8 cores; core i takes split i//2 and
batches {2*(i%2), 2*(i%2)+1} (one split's weights per core, no collectives).

Per-core program, engineered for a dense PE queue (the tensor clock ramps
only under continuous execution, so PE gaps are doubly expensive):
  - All weights resident in SBUF (one DMA each, hoisted).
  - Q/K/V projections run as fp8e4m3 DoubleRow matmuls (K=256/step, 0.5
    cycles/col) with hi+lo compensation (a = hi + lo/16): the lo cross
    terms accumulate in one PSUM bank, are staged out x1/16 (+bias) by DVE,
    and the hi.hi pass lands in a separate ring bank so it never waits;
    the final DVE move adds hi + staged.  This is both ~1.9x faster and
    slightly more accurate than bf16.  Weights are pre-scaled x32 into
    fp8's normal range; the 32x on Q,K is undone by the exp's scale=2^-10
    and the 32x on V by the ones column holding 32 (denominators scale
    with O).  1/sqrt(DK) folded into Wq/bq.
  - V natural [l, e] -> V_aug bf16 with the 32-column per head (col 64).
  - scores^T [lk, lq] per head (K=DK=64, head pairs at PE rows 0/64), causal
    tiles only; exp on ACT -> P^T bf16; triangular mask on diagonal tiles via
    one strided DVE multiply per (head, j).
  - PV in NATURAL orientation: P^T chunks stationary, V_aug moving (65 cols)
    -> O[lq, 64] + softmax denominator (col 64) per lq-tile. Halves PE cost
    vs streaming P^T and lands denominators per-partition: normalize is one
    DVE reciprocal + one free-broadcast multiply (no DMA round-trip).
  - O natural PE-transposed (128x128 tiles, identity) -> Ot [e, l]; DVE
    copies PSUM->SBUF. Out-proj (bf16): Ot stationary x Wo moving ->
    OUT[l,e'] in PSUM; bo' = bv@Wo + bo added by the PSUM->SBUF move from
    a broadcast-DMA'd bias row, then DMA to DRAM.
  - Software pipeline per e-tile window: [Q(et) interleaved with scores of
    pair et-1 head0, PV+normalize of pair et-2, K(et) interleaved with
    scores head1], so ACT exp (~12.1us/pair) hides under ~12.6us of PE work.
    Block 1's V projection and block 0's transpose+out-proj fill the PE
    around the drain of each block's last head pair.
"""

import numpy as np
import ml_dtypes
from contextlib import ExitStack

import concourse.bass as bass
import concourse.bacc as bacc
import concourse.mybir as mybir
import concourse.tile as tile
from concourse.bass_utils import run_bass_kernel_spmd

H = 16
D = 1024
SPLIT = 4
DK = 64
B = 4
SEQ = 4096
L = SEQ // SPLIT          # 1024: tokens per block
NB = 2                    # blocks per core
NCORES = 8
NT = D // 128             # 8 partition tiles of the model dim
NLT = L // 128            # 8 partition tiles of the block length
F32 = mybir.dt.float32
BF16 = mybir.dt.bfloat16
FP8 = mybir.dt.float8e4
EXP = mybir.ActivationFunctionType.Exp
DR = mybir.MatmulPerfMode.DoubleRow
MULT = mybir.AluOpType.mult
ADD = mybir.AluOpType.add


def build_program():
    nc = bacc.Bacc()

    # Q/K/V projections run as fp8e4m3 DoubleRow matmuls (K=256 per step at
    # 0.5 cycles/col) with hi+lo error compensation: a = hi + lo/16, and
    # y = hi.hi + (hi.lo + lo.hi)/16.  Weights are pre-scaled x32 on the host
    # so W lands in fp8's normal range; the 32x on Q,K is absorbed by the
    # exp's scale=2^-10 and the 32x on V by the ones column holding 32.
    # Layouts: k = step*256 + i*128 + p -> stationary [p, 2(i), m],
    # moving [p, 2(i), n] (DoubleRow pairs slot i of both operands).
    xth_d = nc.declare_dram_parameter("xth", [NB, 128, 4, 2, L], FP8, isOutput=False)
    xtl_d = nc.declare_dram_parameter("xtl", [NB, 128, 4, 2, L], FP8, isOutput=False)
    wqh_d = nc.declare_dram_parameter("wqh", [128, NT, 4, 2, 128], FP8, isOutput=False)
    wql_d = nc.declare_dram_parameter("wql", [128, NT, 4, 2, 128], FP8, isOutput=False)
    wkh_d = nc.declare_dram_parameter("wkh", [128, NT, 4, 2, 128], FP8, isOutput=False)
    wkl_d = nc.declare_dram_parameter("wkl", [128, NT, 4, 2, 128], FP8, isOutput=False)
    wvh_d = nc.declare_dram_parameter("wvh", [128, 4, 2, D], FP8, isOutput=False)
    wvl_d = nc.declare_dram_parameter("wvl", [128, 4, 2, D], FP8, isOutput=False)
    wo_d = nc.declare_dram_parameter("wo", [128, NT, D], BF16, isOutput=False)
    bq_d = nc.declare_dram_parameter("bq", [128, NT], F32, isOutput=False)
    bk_d = nc.declare_dram_parameter("bk", [128, NT], F32, isOutput=False)
    bop_d = nc.declare_dram_parameter("bop", [1, D], F32, isOutput=False)
    mask_d = nc.declare_dram_parameter("mask", [128, 128], BF16, isOutput=False)
    id_d = nc.declare_dram_parameter("ident", [128, 128], BF16, isOutput=False)
    out_d = nc.declare_dram_parameter("out", [NB, L, D], F32, isOutput=True)

    with ExitStack() as ctx:
        tc = ctx.enter_context(tile.TileContext(nc))
        consts = ctx.enter_context(tc.tile_pool(name="consts", bufs=1))
        xt_p = ctx.enter_context(tc.tile_pool(name="xt", bufs=1))
        qk_p = ctx.enter_context(tc.tile_pool(name="qk", bufs=3))
        va_p = ctx.enter_context(tc.tile_pool(name="va", bufs=1))
        pt_p = ctx.enter_context(tc.tile_pool(name="pt", bufs=3))
        on_p = ctx.enter_context(tc.tile_pool(name="onat", bufs=1))
        ot_p = ctx.enter_context(tc.tile_pool(name="ot", bufs=1))
        rec_p = ctx.enter_context(tc.tile_pool(name="rec", bufs=1))
        # PSUM: A(3: proj-lo/V/out-proj) + B(3: score tiles) +
        # D(2: proj-hi/PV/transposes) = 8 banks exactly.
        psA = ctx.enter_context(tc.tile_pool(name="psA", bufs=3, space="PSUM"))
        psB = ctx.enter_context(tc.tile_pool(name="psB", bufs=3, space="PSUM"))
        psD = ctx.enter_context(tc.tile_pool(name="psD", bufs=2, space="PSUM"))

        # ---- resident constants / weights ------------------------------
        wqh_sb = consts.tile([128, NT, 4, 2, 128], FP8, tag="wqh")
        wql_sb = consts.tile([128, NT, 4, 2, 128], FP8, tag="wql")
        wkh_sb = consts.tile([128, NT, 4, 2, 128], FP8, tag="wkh")
        wkl_sb = consts.tile([128, NT, 4, 2, 128], FP8, tag="wkl")
        wvh_sb = consts.tile([128, 4, 2, D], FP8, tag="wvh")
        wvl_sb = consts.tile([128, 4, 2, D], FP8, tag="wvl")
        wo_sb = consts.tile([128, NT, D], BF16, tag="wo")
        bq_sb = consts.tile([128, NT], F32, tag="bq")
        bk_sb = consts.tile([128, NT], F32, tag="bk")
        bo_bc = consts.tile([128, D], F32, tag="bo")
        mask_sb = consts.tile([128, 128], BF16, tag="mask")
        id_sb = consts.tile([128, 128], BF16, tag="ident")
        # DMA order follows first use: xt0 + wv-g0 unblock V-proj(0, g0)
        # after ~3MB; everything else streams in under compute.
        xths, xtls = [], []
        xth0 = xt_p.tile([128, 4, 2, L], FP8, name="xth0", tag="xth0")
        xtl0 = xt_p.tile([128, 4, 2, L], FP8, name="xtl0", tag="xtl0")
        # first V-proj pass is (xth stationary) x (wvl moving): load those
        # first, then the second pass's xtl/wvh pieces.
        for lo, hi in ((0, 2), (2, 4)):
            nc.sync.dma_start(out=xth0[:, lo:hi], in_=xth_d[0, :, lo:hi])
            nc.sync.dma_start(out=wvl_sb[:, lo:hi, :, 0:512],
                              in_=wvl_d[:, lo:hi, :, 0:512])
        for lo, hi in ((0, 2), (2, 4)):
            nc.gpsimd.dma_start(out=xtl0[:, lo:hi], in_=xtl_d[0, :, lo:hi])
            nc.gpsimd.dma_start(out=wvh_sb[:, lo:hi, :, 0:512],
                                in_=wvh_d[:, lo:hi, :, 0:512])
        xths.append(xth0)
        xtls.append(xtl0)
        nc.gpsimd.dma_start(out=wqh_sb[:, 0:2], in_=wqh_d[:, 0:2])
        nc.gpsimd.dma_start(out=wql_sb[:, 0:2], in_=wql_d[:, 0:2])
        nc.gpsimd.dma_start(out=wkh_sb[:, 0:2], in_=wkh_d[:, 0:2])
        nc.gpsimd.dma_start(out=wkl_sb[:, 0:2], in_=wkl_d[:, 0:2])
        nc.sync.dma_start(out=bq_sb, in_=bq_d[:, :])
        nc.sync.dma_start(out=bk_sb, in_=bk_d[:, :])
        nc.sync.dma_start(out=mask_sb, in_=mask_d[:, :])
        nc.sync.dma_start(out=wvh_sb[:, :, :, 512:1024],
                          in_=wvh_d[:, :, :, 512:1024])
        nc.sync.dma_start(out=wvl_sb[:, :, :, 512:1024],
                          in_=wvl_d[:, :, :, 512:1024])
        for w_sb, w_d in ((wqh_sb, wqh_d), (wql_sb, wql_d),
                          (wkh_sb, wkh_d), (wkl_sb, wkl_d)):
            nc.sync.dma_start(out=w_sb[:, 2:NT], in_=w_d[:, 2:NT])
        xth1 = xt_p.tile([128, 4, 2, L], FP8, name="xth1", tag="xth1")
        xtl1 = xt_p.tile([128, 4, 2, L], FP8, name="xtl1", tag="xtl1")
        nc.sync.dma_start(out=xth1, in_=xth_d[1])
        nc.sync.dma_start(out=xtl1, in_=xtl_d[1])
        xths.append(xth1)
        xtls.append(xtl1)
        nc.sync.dma_start(out=wo_sb, in_=wo_d[:, :, :])
        nc.sync.dma_start(out=id_sb, in_=id_d[:, :])
        bop_bcast = bass.AP(tensor=bop_d, offset=0, ap=[[0, 128], [1, D]])
        nc.gpsimd.dma_start(out=bo_bc, in_=bop_bcast)

        # V_aug: [128(l within lt), lt, head, 65]; col 64 = ones.
        va = va_p.tile([128, NLT, H, DK + 1], BF16, tag="va")
        # ones column holds 32 so denominators absorb V's 32x weight scale
        nc.gpsimd.memset(va[:, :, :, DK:DK + 1], 32.0)
        # O natural [128(l within lt), lt, et, 128(e within et)]
        onat = on_p.tile([128, NLT, NT, 128], BF16, tag="onat")
        ot = ot_p.tile([128, NT, NLT, 128], BF16, tag="ot")
        # reciprocal softmax denominators [128(lq within lt), head, lt]
        rec = rec_p.tile([128, H, NLT], F32, tag="rec")

        qts, kts, pts = {}, {}, {}

        # ---------------- emission helpers ------------------------------
        def alloc_pt():
            return (pt_p.tile([128, 4, 512], BF16, name="pt0", tag="pt0"),
                    pt_p.tile([128, NLT, 512], BF16, name="pt1", tag="pt1"))

        def scores_units(h, pt_h):
            """Generator: each next() emits one score tile (matmul + exp);
            the per-j triangular mask is emitted with the group's last tile."""
            et, half = h // 2, h % 2
            p0 = 64 * half
            qt_t, kt_t = qts[et], kts[et]
            for j in (0, 1):
                for i in range(4 * j + 4):
                    dg = i - 4 * j
                    lo = 128 * dg if dg > 0 else 0
                    sps = psB.tile([128, 512], F32, name="sps", tag="B")
                    nc.tensor.matmul(
                        sps[:, lo:512],
                        kt_t[p0:p0 + DK, i * 128:(i + 1) * 128],
                        qt_t[p0:p0 + DK, j * 512 + lo:(j + 1) * 512],
                        start=True, stop=True)
                    # scale 2^-10 undoes the 32x on each of Wq and Wk
                    nc.scalar.activation(
                        out=pt_h[j][:, i, lo:512], in_=sps[:, lo:512],
                        func=EXP, scale=float(2.0 ** -10))
                    if i == 4 * j + 3:
                        pt = pt_h[j]
                        diag = bass.AP(
                            tensor=pt.tensor, offset=pt.offset + 2048 * j,
                            ap=[pt.ap[0], [640, 4], [1, 128]])
                        mask_bc = bass.AP(
                            tensor=mask_sb.tensor, offset=mask_sb.offset,
                            ap=[mask_sb.ap[0], [0, 4], [1, 128]])
                        nc.vector.tensor_mul(out=diag, in0=diag, in1=mask_bc)
                    yield

        def emit_proj(blk, et, wh_sb, wl_sb, b_sb, dest, gen):
            """Q or K projection for e-tile et as compensated-fp8 DoubleRow:
            lo cross terms accumulate in an A-ring bank and get staged out
            scaled by 1/16 (+bias); the hi.hi pass runs in a D-ring bank so
            it never waits on the stage.  The final move adds hi + staged.
            Score-tile emission from `gen` is interleaved throughout."""
            xth, xtl = xths[blk], xtls[blk]
            pj = [psA.tile([128, 512], F32, name="psq", tag="A")
                  for _ in range(2)]

            def mm(p, j, w, x, start, stop):
                nc.tensor.matmul(
                    p[j][:, :], w[:, et, st, :, :],
                    x[:, st, :, j * 512:(j + 1) * 512],
                    start=start, stop=stop, perf_mode=DR)

            for st in range(4):
                next(gen, None)
                for j in range(2):
                    mm(pj, j, wh_sb, xtl, st == 0, False)
            for st in range(4):
                next(gen, None)
                for j in range(2):
                    mm(pj, j, wl_sb, xth, False, st == 3)
            stgs = []
            for j in range(2):
                stg = qk_p.tile([128, 512], F32, name="stgq", tag="stgq",
                                bufs=2)
                nc.vector.tensor_scalar(
                    out=stg, in0=pj[j][:, :], scalar1=0.0625,
                    scalar2=b_sb[:, et:et + 1], op0=MULT, op1=ADD)
                stgs.append(stg)
            ph = [psD.tile([128, 512], F32, name="psqh", tag="D")
                  for _ in range(2)]
            for st in range(4):
                next(gen, None)
                for j in range(2):
                    mm(ph, j, wh_sb, xth, st == 0, st == 3)
            for j in range(2):
                nc.vector.tensor_add(
                    out=dest[:, j * 512:(j + 1) * 512],
                    in0=ph[j][:, :], in1=stgs[j])

        def pv_head(h, pt_h, b2s=(0, 1)):
            """PV natural: P^T chunks stationary, V_aug moving (65 cols);
            col 64 accumulates the softmax denominator.  Normalize via
            reciprocal + free-broadcast multiply into O natural."""
            et, half = h // 2, h % 2
            for b2 in b2s:
                opv = psD.tile([128, 4, 128], F32, name="opv", tag="D")
                for m in range(4):
                    jj = 4 * b2 + m
                    c = jj % 4
                    for i in range(jj + 1):
                        nc.tensor.matmul(
                            opv[:, m, 0:DK + 1],
                            pt_h[jj // 4][:, i, 128 * c:128 * (c + 1)],
                            va[:, i, h, :],
                            start=(i == 0), stop=(i == jj))
                nc.vector.reciprocal(
                    out=rec[:, h, 4 * b2:4 * b2 + 4],
                    in_=opv[:, :, DK:DK + 1].rearrange("p a b -> p (a b)"))
                rbc = bass.AP(
                    tensor=rec.tensor,
                    offset=rec.offset + h * NLT + 4 * b2,
                    ap=[rec.ap[0], [1, 4], [0, DK]])
                nc.vector.tensor_mul(
                    out=onat[:, 4 * b2:4 * b2 + 4, et,
                             half * DK:half * DK + DK],
                    in0=opv[:, :, 0:DK], in1=rbc)

        def h_window(blk, et, filler=None):
            if filler is not None:
                filler()
            _h_window(blk, et)

        def _h_window(blk, et):
            """One pipeline window: Q(et) + scores(pair et-1, head0),
            PV+normalize(pair et-2), K(et) + scores(pair et-1, head1)."""
            p1, p2 = et - 1, et - 2
            if 0 <= p1 < NT:
                for hh in (2 * p1, 2 * p1 + 1):
                    pts[hh] = alloc_pt()
                g0 = scores_units(2 * p1, pts[2 * p1])
                g1 = scores_units(2 * p1 + 1, pts[2 * p1 + 1])
            else:
                g0 = g1 = iter(())
            if et < NT:
                qts[et] = qk_p.tile([128, L], BF16, name="qt", tag="qt")
                emit_proj(blk, et, wqh_sb, wql_sb, bq_sb, qts[et], g0)
            else:
                for _ in g0:
                    pass
            if 0 <= p2:
                for hh in (2 * p2, 2 * p2 + 1):
                    pv_head(hh, pts.pop(hh))
            if et < NT:
                kts[et] = qk_p.tile([128, L], BF16, name="kt", tag="kt")
                emit_proj(blk, et, wkh_sb, wkl_sb, bk_sb, kts[et], g1)
            else:
                for _ in g1:
                    pass

        def v_proj_tile(blk, g, lt):
            """V projection, natural [l, e], compensated-fp8 DoubleRow with
            x^T chunks stationary and Wv moving; same stage/overwrite PSUM
            trick as the Q/K projections."""
            xth, xtl = xths[blk], xtls[blk]
            pv = psA.tile([128, 512], F32, name="psv", tag="A")

            def mm(p, x, w, start, stop):
                nc.tensor.matmul(
                    p[:, :], x[:, st, :, lt * 128:(lt + 1) * 128],
                    w[:, st, :, g * 512:(g + 1) * 512],
                    start=start, stop=stop, perf_mode=DR)

            for st in range(4):
                mm(pv, xth, wvl_sb, st == 0, False)
            for st in range(4):
                mm(pv, xtl, wvh_sb, False, st == 3)
            stg = qk_p.tile([128, 512], F32, name="stgv", tag="stgq", bufs=2)
            if blk == 0 or g == 1:
                # block 0's V phase has no exp backlog, and block 1's g1
                # half runs after the drain: ACT is idle in both, and the
                # phase is otherwise DVE-bound.
                nc.scalar.activation(
                    out=stg, in_=pv[:, :],
                    func=mybir.ActivationFunctionType.Identity, scale=0.0625)
            else:
                nc.vector.tensor_scalar_mul(out=stg, in0=pv[:, :],
                                            scalar1=0.0625)
            pvh = psD.tile([128, 512], F32, name="psvh", tag="D")
            for st in range(4):
                mm(pvh, xth, wvh_sb, st == 0, st == 3)
            nc.vector.tensor_add(
                out=va[:, lt, g * 8:(g + 1) * 8, 0:DK],
                in0=pvh.rearrange("p (h k) -> p h k", h=8),
                in1=stg.rearrange("p (h k) -> p h k", h=8))

        def transpose_chunk(et, b2, eng=None):
            """O natural -> Ot for e-tile et, l-tiles [4*b2, 4*b2+4)."""
            tps = psD.tile([128, 4, 128], BF16, name="tps", tag="D")
            for m in range(4):
                nc.tensor.transpose(
                    tps[:, m, :], onat[:, 4 * b2 + m, et, :], id_sb)
            if eng is nc.scalar:
                nc.scalar.activation(
                    out=ot[:, et, 4 * b2:4 * b2 + 4, :], in_=tps,
                    func=mybir.ActivationFunctionType.Copy)
            else:
                nc.vector.tensor_copy(
                    out=ot[:, et, 4 * b2:4 * b2 + 4, :], in_=tps)

        def out_proj_tile(blk, lt, g, halves=1):
            """OUT[l-tile, e'-cols] over et + bias add; DMA SBUF->DRAM.
            halves=2 splits accumulation/add/DMA in two to shorten the
            terminal store chain (used for the program's last tile)."""
            po = psA.tile([128, 512], F32, name="po", tag="A")
            w = 512 // halves
            for hf in range(halves):
                c0 = g * 512 + hf * w
                for et in range(NT):
                    nc.tensor.matmul(
                        po[:, hf * w:(hf + 1) * w], ot[:, et, lt, :],
                        wo_sb[:, et, c0:c0 + w],
                        start=(et == 0), stop=(et == NT - 1))
                osb = qk_p.tile([128, 512], F32, name="osb", tag="osb",
                                bufs=3)
                nc.vector.tensor_add(out=osb[:, 0:w],
                                     in0=po[:, hf * w:(hf + 1) * w],
                                     in1=bo_bc[:, c0:c0 + w])
                nc.sync.dma_start(
                    out=out_d[blk, lt * 128:(lt + 1) * 128, c0:c0 + w],
                    in_=osb[:, 0:w])

        # ---------------- program ---------------------------------------
        for g in (0, 1):
            for lt in range(NLT):
                v_proj_tile(0, g, lt)
        for et in range(NT + 1):
            h_window(0, et)
        drain0 = [(14, pts.pop(14)), (15, pts.pop(15))]

        # block 1 V-proj + block 0 transposes, one chunk per V tile so the
        # DVE copies never bunch; block 0's last-pair exp drains under the
        # g0 half.  et-7 chunks (c=14,15) land after the drain.
        for lt in range(NLT):
            v_proj_tile(1, 0, lt)
            transpose_chunk(lt // 2, lt % 2)
        for b2 in (0, 1):
            for hh, p in drain0:
                pv_head(hh, p, (b2,))
        for lt in range(NLT):
            v_proj_tile(1, 1, lt)
            transpose_chunk((8 + lt) // 2, lt % 2)
        for lt in range(5, NLT - 2):
            out_proj_tile(0, lt, 0)

        # the H windows are ACT-bound (exp); fill their PE slack with the
        # first five po(0, g0) tiles (A-ring is deep enough at bufs=3).
        for et in range(NT + 1):
            h_window(1, et,
                     filler=(lambda lt=et - 4: out_proj_tile(0, lt, 0))
                     if 4 <= et <= 8 else None)
        drain1 = [(14, pts.pop(14)), (15, pts.pop(15))]

        # block 0 out-proj g1, then block 1 transposes, fill the PE while
        # block 1's last-pair exp drains.  All po(0,g1) tiles must precede
        # any tps(1) chunk: both use the shared `ot` buffer.
        for lt in range(NLT):
            out_proj_tile(0, lt, 1)
        for lt in range(NLT - 2, NLT):
            out_proj_tile(0, lt, 0)
        for et in range(7):
            for b2 in range(2):
                # split copies DVE/ACT so neither serializes the batch
                transpose_chunk(et, b2, eng=nc.scalar if b2 else None)
        for b2 in (0, 1):
            for hh, p in drain1:
                pv_head(hh, p, (b2,))
        for b2 in range(2):
            transpose_chunk(7, b2)
        for g in (0, 1):
            for lt in range(NLT):
                out_proj_tile(1, lt, g)
    nc.compile()
    return nc


def _fp8_split(a):
    """a ~ hi + lo/16, both fp8e4m3."""
    np8 = ml_dtypes.float8_e4m3
    hi = a.astype(np8)
    lo = ((a - hi.astype(np.float32)) * np.float32(16.0)).astype(np8)
    return hi, lo


def _dr_moving(a):
    """[K=1024, n] -> DoubleRow moving layout [128(p), 4(step), 2(i), n]
    with k = step*256 + i*128 + p."""
    n = a.shape[1]
    return np.ascontiguousarray(
        a.reshape(4, 2, 128, n).transpose(2, 0, 1, 3))


def _prep_core_inputs(core, x, Wq, Wk, Wv, Wo, bq, bk, bv, bo):
    s = core // 2
    bs = (2 * (core % 2), 2 * (core % 2) + 1)
    sc = np.float32(1.0 / np.sqrt(DK))
    SW = np.float32(32.0)   # weight pre-scale into fp8 normal range
    bf = ml_dtypes.bfloat16
    xth = np.empty((NB, 128, 4, 2, L), ml_dtypes.float8_e4m3)
    xtl = np.empty_like(xth)
    for n, b in enumerate(bs):
        xh, xl = _fp8_split(
            np.ascontiguousarray(x[b, s * L:(s + 1) * L, :].T))
        xth[n], xtl[n] = _dr_moving(xh), _dr_moving(xl)

    def dr_w(W):          # [d, e] -> [128, et, step, 2, 128]
        h, l = _fp8_split(W)
        lay = lambda a: np.ascontiguousarray(
            a.reshape(4, 2, 128, NT, 128).transpose(2, 3, 0, 1, 4))
        return lay(h), lay(l)

    wqh, wql = dr_w(Wq[s] * (sc * SW))
    wkh, wkl = dr_w(Wk[s] * SW)
    wvh_f, wvl_f = _fp8_split(Wv[s] * SW)
    wvh, wvl = _dr_moving(wvh_f), _dr_moving(wvl_f)
    wo = np.ascontiguousarray(
        Wo[s].reshape(NT, 128, D).transpose(1, 0, 2)).astype(bf)
    bqt = np.ascontiguousarray(
        (bq[s] * sc * SW).reshape(NT, 128).T).astype(np.float32)
    bkt = np.ascontiguousarray(
        (bk[s] * SW).reshape(NT, 128).T).astype(np.float32)
    bop = (bv[s] @ Wo[s] + bo[s]).reshape(1, D).astype(np.float32)
    mask = np.triu(np.ones((128, 128))).astype(bf)
    ident = np.eye(128).astype(bf)
    return {"xth": xth, "xtl": xtl, "wqh": wqh, "wql": wql,
            "wkh": wkh, "wkl": wkl, "wvh": wvh, "wvl": wvl, "wo": wo,
            "bq": bqt, "bk": bkt, "bop": bop, "mask": mask, "ident": ident}


_PROGRAM_CACHE = {}


def run(x, Wq, Wk, Wv, Wo, bq, bk, bv, bo, trace=False, **run_kwargs):
    x = np.asarray(x, np.float32)
    Wq, Wk, Wv, Wo = (np.asarray(a, np.float32) for a in (Wq, Wk, Wv, Wo))
    bq, bk, bv, bo = (np.asarray(a, np.float32) for a in (bq, bk, bv, bo))

    if "nc" not in _PROGRAM_CACHE:
        _PROGRAM_CACHE["nc"] = build_program()
    nc = _PROGRAM_CACHE["nc"]

    in_maps = [_prep_core_inputs(c, x, Wq, Wk, Wv, Wo, bq, bk, bv, bo)
               for c in range(NCORES)]
    res = run_bass_kernel_spmd(nc, in_maps, core_ids=list(range(NCORES)),
                               trace=trace, **run_kwargs)
    out = np.empty((B, SEQ, D), np.float32)
    for c in range(NCORES):
        s = c // 2
        for n, b in enumerate((2 * (c % 2), 2 * (c % 2) + 1)):
            out[b, s * L:(s + 1) * L, :] = res.results[c]["out"][n]
    return out, res


def kernel(x, Wq, Wk, Wv, Wo, bq, bk, bv, bo):
    out, _ = run(x, Wq, Wk, Wv, Wo, bq, bk, bv, bo, trace=False)
    return out
